# revision 1
# baseline (speedup 1.0000x reference)
"""Trainium2 Bass kernel for the mention/context attention + label head model.

Sharding: data-parallel over the mention batch B=512 (64 mentions/core) for
the attention stages; the 10331-label output head is sharded over the label
dim (1292 labels/core) with an on-device AllGather of the pooled
representations. Host does input layout prep (gather, transposes) and output
concat only.
"""
import sys
sys.path.insert(0, "/opt/trn_rl_repo")

import numpy as np
from contextlib import ExitStack

import concourse.bass as bass
import concourse.bacc as bacc
import concourse.tile as tile
from concourse import mybir
from concourse.bass_utils import run_bass_kernel_spmd
from concourse.masks import make_identity

F32 = mybir.dt.float32
F32R = mybir.dt.float32r
AF = mybir.ActivationFunctionType

NCORES = 8
N_SENT, B, S, D = 256, 512, 128, 1024
L, LAT = 10331, 101
BC = B // NCORES            # 64 mentions per core
KC = D // 128               # 8 contraction chunks
TOK = BC * S                # 8192 tokens per core
NT = 512                    # token tile (matmul free dim)
NJ = TOK // NT              # 16 token tiles
MPJ = NT // S               # 4 mentions per token tile
D2 = 2 * D
D2C = D2 // 128             # 16
LI = 1292                   # padded label slice per core (8*1292 = 10336 >= L)
LPAD = NCORES * LI
LCH = (LI + 127) // 128     # 11 label chunks (last partial: 12)

_CACHE = {}


def _build():
    nc = bacc.Bacc("TRN2", num_devices=NCORES, debug=False)

    xt_d = nc.dram_tensor("xt", [KC, 128, TOK], F32R, kind="ExternalInput").ap()
    x_d = nc.dram_tensor("x", [BC, S, D], F32R, kind="ExternalInput").ap()
    mm_d = nc.dram_tensor("mmask", [NJ, NT], F32, kind="ExternalInput").ap()
    cm_d = nc.dram_tensor("cmask", [NJ, NT], F32, kind="ExternalInput").ap()
    dist_d = nc.dram_tensor("dist", [NJ, NT], F32R, kind="ExternalInput").ap()
    wmT_d = nc.dram_tensor("wmT", [KC, 128, D], F32R, kind="ExternalInput").ap()
    wcT_d = nc.dram_tensor("wcT", [KC, 128, D], F32R, kind="ExternalInput").ap()
    wmcT_d = nc.dram_tensor("wmcT", [KC, 128, D], F32R, kind="ExternalInput").ap()
    womr_d = nc.dram_tensor("womr", [128, KC, BC], F32R, kind="ExternalInput").ap()
    wocr_d = nc.dram_tensor("wocr", [128, KC, BC], F32R, kind="ExternalInput").ap()
    wd_d = nc.dram_tensor("wd", [1, D], F32R, kind="ExternalInput").ap()
    woT_d = nc.dram_tensor("woT", [D2C, 128, LI], F32R, kind="ExternalInput").ap()
    wf2lT_d = nc.dram_tensor("wf2lT", [D2C, 128, LAT], F32R, kind="ExternalInput").ap()
    wl2lT_d = nc.dram_tensor("wl2lT", [LAT, LI], F32R, kind="ExternalInput").ap()
    lsc_d = nc.dram_tensor("lsc", [1, 1], F32, kind="ExternalInput").ap()
    outT_d = nc.dram_tensor("outT", [LI, B], F32, kind="ExternalOutput").ap()
    olatT_d = nc.dram_tensor("olatT", [LI, B], F32, kind="ExternalOutput").ap()

    with tile.TileContext(nc) as tc, ExitStack() as ctx:
        # ---- global pools ----
        pers = ctx.enter_context(tc.tile_pool(name="pers", bufs=1))
        bigp = ctx.enter_context(tc.tile_pool(name="bigp", bufs=2, space="PSUM"))
        spsum = ctx.enter_context(tc.tile_pool(name="spsum", bufs=2, space="PSUM"))
        wpsum = ctx.enter_context(tc.tile_pool(name="wpsum", bufs=2, space="PSUM"))
        tpsum = ctx.enter_context(tc.tile_pool(name="tpsum", bufs=2, space="PSUM"))

        ident_f = pers.tile([128, 128], F32)
        make_identity(nc, ident_f)
        ident = pers.tile([128, 128], F32R)
        nc.vector.tensor_copy(ident, ident_f)
        zcol = pers.tile([128, BC], F32R)
        nc.vector.tensor_scalar_mul(zcol, ident_f[:, 0:BC], 0.0)

        worep = {}
        worep["men"] = pers.tile([128, KC, BC], F32R, tag="worep_men", name="worep_men")
        nc.sync.dma_start(out=worep["men"], in_=womr_d)
        worep["ctx"] = pers.tile([128, KC, BC], F32R, tag="worep_ctx", name="worep_ctx")
        nc.sync.dma_start(out=worep["ctx"], in_=wocr_d)
        wd_sb = pers.tile([1, D], F32R)
        nc.sync.dma_start(out=wd_sb, in_=wd_d)

        mask_d = {"men": mm_d, "ctx": cm_d}

        mrT = pers.tile([128, KC, BC], F32R)   # men_repr^T   [d, b]
        crT = pers.tile([128, KC, BC], F32R)   # ctx_repr^T   [d, b]
        t2 = pers.tile([128, KC, BC], F32)     # (W_ctx_m @ men_repr^T)^T->[e,b]

        def attention_pass(which, w_dram, dst):
            with tc.tile_pool(name="xt_" + which, bufs=2) as xtp, \
                 tc.tile_pool(name="x_" + which, bufs=2) as xp, \
                 tc.tile_pool(name="h_" + which, bufs=2) as hp, \
                 tc.tile_pool(name="w_" + which, bufs=1) as wp, \
                 tc.tile_pool(name="sm_" + which, bufs=6) as smp, \
                 tc.tile_pool(name="ml_" + which, bufs=8) as mlp, \
                 tc.tile_pool(name="dj_" + which, bufs=2) as djp, \
                 tc.tile_pool(name="r_" + which, bufs=1) as rp:
                w_sb = wp.tile([128, KC, D], F32R)
                for k in range(KC):
                    nc.sync.dma_start(out=w_sb[:, k, :], in_=w_dram[k])
                psw0 = wpsum.tile([BC, NT], F32, tag="psw")
                psw1 = wpsum.tile([BC, NT], F32, tag="psw")
                for j in range(NJ):
                    xt_j = xtp.tile([128, KC, NT], F32R, tag="xt")
                    for k in range(KC):
                        nc.sync.dma_start(
                            out=xt_j[:, k, :], in_=xt_d[k, :, j * NT:(j + 1) * NT])
                    x_j = xp.tile([128, MPJ, D], F32R, tag="x")
                    nc.sync.dma_start(
                        out=x_j,
                        in_=x_d[j * MPJ:(j + 1) * MPJ].rearrange("b s d -> s b d"))
                    if which == "ctx":
                        dist_j = djp.tile([1, NT], F32R, tag="dj")
                        nc.sync.dma_start(out=dist_j, in_=dist_d[j:j + 1, :])
                    h_j = hp.tile([128, KC, NT], F32R, tag="h")
                    for m in range(KC):
                        ps = bigp.tile([128, NT], F32, tag="big")
                        for k in range(KC):
                            nc.tensor.matmul(
                                ps, w_sb[:, k, m * 128:(m + 1) * 128], xt_j[:, k, :],
                                start=(k == 0),
                                stop=(k == KC - 1 and which == "men"))
                        if which == "ctx":
                            nc.tensor.matmul(
                                ps, wd_sb[0:1, m * 128:(m + 1) * 128], dist_j,
                                start=False, stop=True)
                            # add t2[e, b] broadcast along s
                            t2b = bass.AP(
                                tensor=t2.tensor,
                                offset=t2[:, m, j * MPJ].offset,
                                ap=[list(t2.ap[0]), [1, MPJ], [0, S]])
                            ps3 = ps.rearrange("p (b s) -> p b s", b=MPJ)
                            nc.vector.tensor_add(ps3, ps3, t2b)
                        nc.scalar.activation(h_j[:, m, :], ps, AF.Tanh)
                    # scores, replicated across 64 partitions
                    pss = spsum.tile([BC, NT], F32, tag="pss")
                    for m in range(KC):
                        nc.tensor.matmul(
                            pss, worep[which][:, m, :], h_j[:, m, :],
                            start=(m == 0), stop=(m == KC - 1))
                    nm = smp.tile([1, NT], F32, tag="nm")
                    nc.sync.dma_start(out=nm, in_=mask_d[which][j:j + 1, :])
                    # (mask*1e4 - 1e4) == -1e4*(1-mask)
                    nc.vector.tensor_scalar(
                        nm, nm, 10000.0, 10000.0,
                        mybir.AluOpType.mult, mybir.AluOpType.subtract)
                    sc = smp.tile([1, NT], F32, tag="sc")
                    nc.vector.tensor_add(sc, pss[0:1, :], nm)
                    sc3 = sc.rearrange("p (b s) -> p b s", b=MPJ)
                    mx = smp.tile([1, MPJ], F32, tag="mx")
                    nc.vector.tensor_reduce(
                        mx, sc3, axis=mybir.AxisListType.X, op=mybir.AluOpType.max)
                    mxb = bass.AP(
                        tensor=mx.tensor, offset=mx.offset,
                        ap=[list(mx.ap[0]), [1, MPJ], [0, S]])
                    nc.vector.tensor_tensor(
                        sc3, sc3, mxb, op=mybir.AluOpType.subtract)
                    ex = smp.tile([1, NT], F32, tag="ex")
                    nc.scalar.activation(ex, sc, AF.Exp)
                    ex3 = ex.rearrange("p (b s) -> p b s", b=MPJ)
                    sm = smp.tile([1, MPJ], F32, tag="sm")
                    nc.vector.tensor_reduce(
                        sm, ex3, axis=mybir.AxisListType.X, op=mybir.AluOpType.add)
                    rc = smp.tile([1, MPJ], F32, tag="rc")
                    nc.vector.reciprocal(rc, sm)
                    rcb = bass.AP(
                        tensor=rc.tensor, offset=rc.offset,
                        ap=[list(rc.ap[0]), [1, MPJ], [0, S]])
                    at = smp.tile([1, NT], F32, tag="at")
                    at3 = at.rearrange("p (b s) -> p b s", b=MPJ)
                    nc.vector.tensor_tensor(at3, ex3, rcb, op=mybir.AluOpType.mult)
                    atT = smp.tile([128, MPJ], F32R, tag="atT")
                    for r in range(MPJ):
                        pst = tpsum.tile([128, 1], F32, tag="tp")
                        nc.tensor.transpose(
                            pst, at[0:1, r * S:(r + 1) * S], ident_f[0:1, 0:1])
                        nc.vector.tensor_copy(atT[:, r:r + 1], pst)
                    # weighted sums: one masked-column lhsT per mention
                    for r in range(MPJ):
                        b = j * MPJ + r
                        ml = mlp.tile([128, BC], F32R, tag="ml")
                        nc.vector.tensor_copy(ml, zcol)
                        nc.vector.tensor_copy(ml[:, b:b + 1], atT[:, r:r + 1])
                        nc.tensor.matmul(
                            psw0, ml, x_j[:, r, 0:512],
                            start=(b == 0), stop=(b == BC - 1),
                            skip_group_check=True)
                        nc.tensor.matmul(
                            psw1, ml, x_j[:, r, 512:1024],
                            start=(b == 0), stop=(b == BC - 1),
                            skip_group_check=True)
                # evacuate weighted sums -> R [64, 1024] -> transpose -> dst
                R = rp.tile([BC, D], F32R)
                nc.scalar.activation(R[:, 0:512], psw0, AF.Copy)
                nc.scalar.activation(R[:, 512:1024], psw1, AF.Copy)
                for k in range(KC):
                    pst = tpsum.tile([128, BC], F32R, tag="tp")
                    nc.tensor.transpose(
                        pst, R[:, k * 128:(k + 1) * 128], ident[0:BC, 0:BC])
                    nc.vector.tensor_copy(dst[:, k, :], pst)

        attention_pass("men", wmT_d, mrT)

        # t2 = (W_ctx_m @ men_repr)^T : first compute t2T [b, e] then transpose
        with tc.tile_pool(name="wmcp", bufs=2) as wmcp, \
             tc.tile_pool(name="t2tp", bufs=1) as t2tp:
            p0 = wpsum.tile([BC, 512], F32, tag="psw")
            p1 = wpsum.tile([BC, 512], F32, tag="psw")
            for k in range(KC):
                wmc_k = wmcp.tile([128, D], F32R, tag="wmc")
                nc.sync.dma_start(out=wmc_k, in_=wmcT_d[k])
                nc.tensor.matmul(p0, mrT[:, k, :], wmc_k[:, 0:512],
                                 start=(k == 0), stop=(k == KC - 1))
                nc.tensor.matmul(p1, mrT[:, k, :], wmc_k[:, 512:1024],
                                 start=(k == 0), stop=(k == KC - 1))
            t2T = t2tp.tile([BC, D], F32R)
            nc.scalar.activation(t2T[:, 0:512], p0, AF.Copy)
            nc.scalar.activation(t2T[:, 512:1024], p1, AF.Copy)
            for k in range(KC):
                pst = tpsum.tile([128, BC], F32R, tag="tp")
                nc.tensor.transpose(
                    pst, t2T[:, k * 128:(k + 1) * 128], ident[0:BC, 0:BC])
                nc.vector.tensor_copy(t2[:, k, :], pst)

        attention_pass("ctx", wcT_d, crT)

        # ---- all-gather final_repr^T across cores ----
        with tc.tile_pool(name="dram", bufs=1, space="DRAM") as dram:
            frT_loc = dram.tile([D2C, 128, BC], F32R)
            frT_all = dram.tile([NCORES, D2C, 128, BC], F32R)
            nc.sync.dma_start(
                out=frT_loc[0:KC].rearrange("k p b -> p k b"), in_=mrT)
            nc.sync.dma_start(
                out=frT_loc[KC:D2C].rearrange("k p b -> p k b"), in_=crT)
            nc.gpsimd.collective_compute(
                "AllGather", mybir.AluOpType.bypass,
                replica_groups=[list(range(NCORES))],
                ins=[frT_loc.opt()], outs=[frT_all.opt()])

            # ---- head ----
            with tc.tile_pool(name="hd", bufs=1) as hd, \
                 tc.tile_pool(name="wop", bufs=2) as wop, \
                 tc.tile_pool(name="osb", bufs=2) as osbp:
                frg = hd.tile([128, D2C, NCORES, BC], F32R)
                for r in range(NCORES):
                    nc.sync.dma_start(
                        out=frg[:, :, r, :],
                        in_=frT_all[r].rearrange("k p b -> p k b"))
                frg2 = frg.rearrange("p k r b -> p k (r b)")

                wf2l_sb = hd.tile([128, D2C, LAT], F32R)
                for k2 in range(D2C):
                    nc.sync.dma_start(out=wf2l_sb[:, k2, :], in_=wf2lT_d[k2])
                latp = bigp.tile([LAT, B], F32, tag="big")
                for k2 in range(D2C):
                    nc.tensor.matmul(latp, wf2l_sb[:, k2, :], frg2[:, k2, :],
                                     start=(k2 == 0), stop=(k2 == D2C - 1))
                lat_sb = hd.tile([LAT, B], F32R)
                nc.scalar.activation(lat_sb, latp, AF.Copy)

                wl2lu = hd.tile([LAT, LI], F32R)
                nc.sync.dma_start(out=wl2lu, in_=wl2lT_d)
                lscp = hd.tile([1, 1], F32)
                nc.sync.dma_start(out=lscp, in_=lsc_d)
                lsc_bc = hd.tile([LAT, 1], F32)
                nc.gpsimd.partition_broadcast(lsc_bc, lscp)
                wl2ls = hd.tile([LAT, LI], F32R)
                nc.vector.tensor_scalar_mul(wl2ls, wl2lu, lsc_bc)

                for lc in range(LCH):
                    mlen = min(128, LI - lc * 128)
                    wo_t = wop.tile([128, D2C, 128], F32R, tag="wo")
                    for k2 in range(D2C):
                        nc.sync.dma_start(
                            out=wo_t[:, k2, 0:mlen],
                            in_=woT_d[k2, :, lc * 128:lc * 128 + mlen])
                    pso = bigp.tile([128, B], F32, tag="big")
                    for k2 in range(D2C):
                        nc.tensor.matmul(
                            pso[0:mlen], wo_t[:, k2, 0:mlen], frg2[:, k2, :],
                            start=(k2 == 0), stop=False, skip_group_check=True)
                    nc.tensor.matmul(
                        pso[0:mlen], wl2ls[:, lc * 128:lc * 128 + mlen], lat_sb,
                        start=False, stop=True, skip_group_check=True)
                    osb = osbp.tile([128, B], F32, tag="osb")
                    nc.scalar.activation(osb[0:mlen], pso[0:mlen], AF.Copy)
                    nc.sync.dma_start(
                        out=outT_d[lc * 128:lc * 128 + mlen, :], in_=osb[0:mlen])
                    psol = spsum.tile([128, B], F32, tag="pss")
                    nc.tensor.matmul(
                        psol[0:mlen], wl2lu[:, lc * 128:lc * 128 + mlen], lat_sb,
                        start=True, stop=True)
                    olsb = osbp.tile([128, B], F32, tag="olsb")
                    nc.scalar.activation(olsb[0:mlen], psol[0:mlen], AF.Copy)
                    nc.sync.dma_start(
                        out=olatT_d[lc * 128:lc * 128 + mlen, :], in_=olsb[0:mlen])

    nc.compile()
    return nc


def _prep(inputs):
    f = np.float32
    elmo = np.asarray(inputs["elmo_outputs"], f)
    men_mask = np.asarray(inputs["men_mask"], f)
    ctx_mask = np.asarray(inputs["ctx_mask"], f)
    dist = np.asarray(inputs["dist"], f)
    gathers = np.asarray(inputs["gathers"])
    W_men_m = np.asarray(inputs["W_men_m"], f)
    W_men_o = np.asarray(inputs["W_men_o"], f).reshape(-1)
    W_ctx_c = np.asarray(inputs["W_ctx_c"], f)
    W_ctx_m = np.asarray(inputs["W_ctx_m"], f)
    w_ctx_d = np.asarray(inputs["w_ctx_d"], f).reshape(-1)
    W_ctx_o = np.asarray(inputs["W_ctx_o"], f).reshape(-1)
    W_out = np.asarray(inputs["W_out"], f)
    W_f2l = np.asarray(inputs["W_f2l"], f)
    W_l2l = np.asarray(inputs["W_l2l"], f)
    lsc = np.asarray(inputs["latent_scalar"], f).reshape(1, 1)

    # shared (replicated) weight layouts
    wmT = np.ascontiguousarray(W_men_m.T.reshape(KC, 128, D))
    wcT = np.ascontiguousarray(W_ctx_c.T.reshape(KC, 128, D))
    wmcT = np.ascontiguousarray(W_ctx_m.T.reshape(KC, 128, D))
    womr = np.ascontiguousarray(
        np.repeat(W_men_o.reshape(KC, 128).T[:, :, None], BC, axis=2))
    wocr = np.ascontiguousarray(
        np.repeat(W_ctx_o.reshape(KC, 128).T[:, :, None], BC, axis=2))
    wd = np.ascontiguousarray(w_ctx_d.reshape(1, D))
    woT_pad = np.zeros((D2, LPAD), f)
    woT_pad[:, :L] = W_out.T
    wf2lT = np.ascontiguousarray(W_f2l.T.reshape(D2C, 128, LAT))
    wl2lT_pad = np.zeros((LAT, LPAD), f)
    wl2lT_pad[:, :L] = W_l2l.T

    in_maps = []
    for i in range(NCORES):
        g = gathers[i * BC:(i + 1) * BC]
        xb = elmo[g]                                   # [64, 128, 1024]
        xt = np.ascontiguousarray(
            xb.reshape(TOK, D).T.reshape(KC, 128, TOK))
        mm = np.ascontiguousarray(men_mask[i * BC:(i + 1) * BC].reshape(NJ, NT))
        cm = np.ascontiguousarray(ctx_mask[i * BC:(i + 1) * BC].reshape(NJ, NT))
        in_maps.append({
            "xt": xt,
            "x": np.ascontiguousarray(xb),
            "mmask": mm,
            "cmask": cm,
            "dist": np.ascontiguousarray(
                dist[i * BC:(i + 1) * BC].reshape(NJ, NT)),
            "wmT": wmT, "wcT": wcT, "wmcT": wmcT,
            "womr": womr, "wocr": wocr, "wd": wd,
            "woT": np.ascontiguousarray(
                woT_pad[:, i * LI:(i + 1) * LI].reshape(D2C, 128, LI)),
            "wf2lT": wf2lT,
            "wl2lT": np.ascontiguousarray(wl2lT_pad[:, i * LI:(i + 1) * LI]),
            "lsc": lsc,
        })
    return in_maps


def kernel(**inputs):
    if "nc" not in _CACHE:
        _CACHE["nc"] = _build()
    nc = _CACHE["nc"]
    in_maps = _prep(inputs)
    res = run_bass_kernel_spmd(nc, in_maps, core_ids=list(range(NCORES)))
    outs = res.results
    outT = np.concatenate([outs[i]["outT"] for i in range(NCORES)], axis=0)
    outputs = np.ascontiguousarray(outT[:L].T).astype(np.float32)
    olatT = np.concatenate([outs[i]["olatT"] for i in range(NCORES)], axis=0)
    outputs_latent = np.ascontiguousarray(olatT[:L].T).astype(np.float32)
    return outputs, outputs_latent

